# revision 10
# baseline (speedup 1.0000x reference)
"""Trainium2 Bass kernel for nn_DecoderLayer (GNN message passing decoder layer).

Math (per reference):
  seq_j = seq_emb[edge_idx] * ar_mask[..., None]
  x = concat([h_i, h_j, edge_h, seq_j], -1)            # [res,k,4h]
  msg = gelu(x @ mW1 + mb1); msg = gelu(msg @ mW2 + mb2); msg = msg @ mW3 + mb3
  agg = msg.sum(1)
  h = LN(node_h + agg) * g1 + b1
  ff = gelu(h @ fW1 + fb1) @ fW2 + fb2
  h = LN(h + ff) * g2 + b2

Strategy (8-way data parallel over the residue dim, no collectives):
  - mm1 is decomposed: x@mW1 = h_i@Wa + h_j@Wb + edge_h@Wc + seq_j@Wd.
    h_j@Wb and seq_emb@Wd are precomputed per *global* node into a fused
    bf16 gather table [8192, 256]; per-edge rows are fetched with
    dma_gather (8 chunks of 6144 rows, round-robin over 4 SWDGE queues —
    big chunks + multi-queue cut descriptor-gen/ring-wait ~4x).
  - ar_mask is applied with ONE strided broadcast tensor_mul per chunk
    (stride-0 inner dim), then the two gathered halves are pre-added on
    DVE so each 128-edge sub-block needs a single identity-matmul PSUM
    inject instead of two (saves 384 PE cols + 3 LDWEIGHTS per tile).
  - k-reduction is moved before mm3 (linearity): 48x less mm3 work; the
    k-sum itself is one big tensor_reduce per chunk.
  - activations live feature-major ("T layout", [feat, rows]) so every
    matmul uses weights as lhsT directly with zero transposes.
  - edge_h / node_h / seq_emb are host-cast to bf16 so all phase-2 DMA
    loads run on HWDGE engines (no gpsimd cast DMAs) at half the bytes.
  - DMA queue split: big input streams ride the Scalar HWDGE queue, the
    Sync queue stays shallow for table writes / small singles / outputs
    (a single in-order queue serialized phase 1 behind MB-scale loads).
  - phase 3 is interleaved: each chunk's k-sum feeds mm3+LN1 for exactly
    one 128-node block; the FFN + LN2 + output run per 512-node half as
    soon as their 4 blocks are ready, hiding nearly all of phase 3.
"""

import os
import sys

sys.path.insert(0, "/opt/trn_rl_repo")

import numpy as np
import ml_dtypes

import concourse.bacc as bacc
import concourse.bass as bass
import concourse.mybir as mybir
import concourse.tile as tile
from concourse import bass_utils

BF16 = ml_dtypes.bfloat16
F32 = mybir.dt.float32
BF = mybir.dt.bfloat16
I16 = mybir.dt.int16
F8 = mybir.dt.float8e4
F8NP = ml_dtypes.float8_e4m3fn

RES, KK, H = 8192, 48, 128
N_CORES = 8
RT = 384  # rows per psum tile (8 nodes x 48 edges)
N_CH = 8  # gather chunks per core
NQ = 4  # SWDGE queues


def build_nc(n_glob, n_loc, num_devices):
    """Build the bass program for one core holding n_loc nodes of an
    n_glob-node graph. All sizes in nodes; n_loc % 128 == 0, n_glob % 128 == 0."""
    E = n_loc * KK
    assert E % RT == 0
    T = E // RT  # number of 384-row tiles
    assert T % N_CH == 0
    cht = T // N_CH  # tiles per chunk
    ch_rows = cht * RT  # rows per chunk (gather num_idxs)
    sub_ch = ch_rows // 128  # 128-row sub-blocks per chunk
    nblk = n_loc // 128  # node blocks
    gblk = n_glob // 128  # global node blocks (table build)
    assert nblk == N_CH  # one LN1 block per chunk
    nh_half = max(1, n_loc // 512)  # FF halves of <=512 nodes
    hw_ = min(512, n_loc)  # nodes per half
    blk_h = hw_ // 128  # blocks per half

    nc = bacc.Bacc("TRN2", target_bir_lowering=False, debug=False,
                   num_devices=num_devices, num_swdge_queues=NQ)

    def din(name, shape, dt):
        return nc.dram_tensor(name, shape, dt, kind="ExternalInput")

    edge_hT = din("edge_hT", [H, E], F8)
    idx16 = din("idx16", [16, E // 16], I16)
    maskc = din("maskc", [128, 3 * T], F32)
    node_hT = din("node_hT", [H, n_glob], F8)
    seqT = din("seqT", [H, n_glob], F8)
    nhl = din("nhl", [128, nblk, H], F32)
    nhTl = din("nhTl", [128, n_loc], BF)
    wa = din("wa", [H, H], BF)
    wb = din("wb", [H, H], BF)
    wc = din("wc", [H, H], BF)
    wd = din("wd", [H, H], BF)
    w2 = din("w2", [H, H], BF)
    w3 = din("w3", [H, H], BF)
    fw1 = din("fw1", [H, 4 * H], BF)
    fw2 = din("fw2", [H, 4, H], BF)
    ident = din("ident", [128, 128], BF)
    mb1c = din("mb1c", [H, 1], F32)
    mb2c = din("mb2c", [H, 1], F32)
    mb3x48 = din("mb3x48", [H, 1], F32)
    fb1c = din("fb1c", [H, 4], F32)
    fb2c = din("fb2c", [H, 1], F32)
    g1bc = din("g1bc", [128, H], BF)
    b1bc = din("b1bc", [128, H], BF)
    g2bc = din("g2bc", [128, H], F32)
    b2bc = din("b2bc", [128, H], F32)
    out = nc.dram_tensor("out", [n_loc, H], F32, kind="ExternalOutput")

    GELU = mybir.ActivationFunctionType.Gelu
    IDENT = mybir.ActivationFunctionType.Identity
    COPY = mybir.ActivationFunctionType.Copy
    SQRT = mybir.ActivationFunctionType.Sqrt
    AX = mybir.AxisListType.X
    SUB = mybir.AluOpType.subtract
    MUL = mybir.AluOpType.mult

    with tile.TileContext(nc) as tc:
        with tc.tile_pool(name="singles", bufs=1) as sg, \
             tc.tile_pool(name="dram", bufs=1, space="DRAM") as dp:
            # ---- resident tiles ----
            # big input streams on the Scalar HWDGE queue
            s_idx = sg.tile([128, E // 16], I16)
            idx_bc = bass.AP(tensor=idx16, offset=0,
                             ap=[[0, 8], [E // 16, 16], [1, E // 16]])
            nc.scalar.dma_start(out=s_idx[:], in_=idx_bc)
            s_maskc = sg.tile([128, 3 * T], F32)
            nc.scalar.dma_start(out=s_maskc[:], in_=maskc.ap())
            s_nhl = sg.tile([128, nblk, H], F32)
            nc.scalar.dma_start(out=s_nhl[:], in_=nhl.ap())
            s_nhTl = sg.tile([128, n_loc], BF)
            nc.scalar.dma_start(out=s_nhTl[:], in_=nhTl.ap())
            # small singles on the Sync queue
            s_wa = sg.tile([H, H], BF)
            nc.sync.dma_start(out=s_wa[:], in_=wa.ap())
            s_wb = sg.tile([H, H], BF)
            nc.sync.dma_start(out=s_wb[:], in_=wb.ap())
            s_wc = sg.tile([H, H], BF)
            nc.sync.dma_start(out=s_wc[:], in_=wc.ap())
            s_wd = sg.tile([H, H], BF)
            nc.sync.dma_start(out=s_wd[:], in_=wd.ap())
            s_w2 = sg.tile([H, H], BF)
            nc.sync.dma_start(out=s_w2[:], in_=w2.ap())
            s_w3 = sg.tile([H, H], BF)
            nc.sync.dma_start(out=s_w3[:], in_=w3.ap())
            s_fw1 = sg.tile([H, 4 * H], BF)
            nc.sync.dma_start(out=s_fw1[:], in_=fw1.ap())
            s_fw2 = sg.tile([H, 4, H], BF)
            nc.sync.dma_start(out=s_fw2[:], in_=fw2.ap())
            s_id = sg.tile([128, 128], BF)
            nc.sync.dma_start(out=s_id[:], in_=ident.ap())
            s_mb1c = sg.tile([H, 1], F32)
            nc.sync.dma_start(out=s_mb1c[:], in_=mb1c.ap())
            s_mb2c = sg.tile([H, 1], F32)
            nc.sync.dma_start(out=s_mb2c[:], in_=mb2c.ap())
            s_mb3x48 = sg.tile([H, 1], F32)
            nc.sync.dma_start(out=s_mb3x48[:], in_=mb3x48.ap())
            s_fb1c = sg.tile([H, 4], F32)
            nc.sync.dma_start(out=s_fb1c[:], in_=fb1c.ap())
            s_fb2c = sg.tile([H, 1], F32)
            nc.sync.dma_start(out=s_fb2c[:], in_=fb2c.ap())
            s_g1bc = sg.tile([128, H], BF)
            nc.sync.dma_start(out=s_g1bc[:], in_=g1bc.ap())
            s_b1bc = sg.tile([128, H], BF)
            nc.sync.dma_start(out=s_b1bc[:], in_=b1bc.ap())
            s_g2bc = sg.tile([128, H], F32)
            nc.sync.dma_start(out=s_g2bc[:], in_=g2bc.ap())
            s_b2bc = sg.tile([128, H], F32)
            nc.sync.dma_start(out=s_b2bc[:], in_=b2bc.ap())
            s_eps = sg.tile([128, 1], F32)
            nc.vector.memset(s_eps[:], 1e-5)

            s_aggT = sg.tile([128, n_loc], F32)
            s_aggTb = sg.tile([128, n_loc], BF)
            s_a2Tb = sg.tile([128, n_loc], BF)
            s_h1T = sg.tile([128, n_loc], BF)
            s_h1rm = sg.tile([128, nblk, H], BF)

            table = dp.tile([n_glob, 256], F8)

            # ---- phase 1: gather table precompute (2 blocks per psum) ----
            with tc.tile_pool(name="p1s", bufs=4) as p1s, \
                 tc.tile_pool(name="p1p", bufs=2, space="PSUM") as p1p:
                s_nhT = p1s.tile([128, n_glob], F8, tag="nhT")
                nc.scalar.dma_start(out=s_nhT[:], in_=node_hT.ap())
                s_seT = p1s.tile([128, n_glob], F8, tag="seT")
                nc.scalar.dma_start(out=s_seT[:], in_=seqT.ap())
                for bp in range(gblk // 4):
                    b0 = 4 * bp
                    ps = p1p.tile([128, 4, 256], F32, tag="tps")
                    for k in range(4):
                        b = b0 + k
                        nc.tensor.matmul(out=ps[:, k, 0:128],
                                         lhsT=s_nhT[:, 128 * b:128 * (b + 1)],
                                         rhs=s_wb[:], start=True, stop=True)
                        nc.tensor.matmul(out=ps[:, k, 128:256],
                                         lhsT=s_seT[:, 128 * b:128 * (b + 1)],
                                         rhs=s_wd[:], start=True, stop=True)
                    tb = p1s.tile([128, 4, 256], F8, tag="tb")
                    nc.scalar.activation(out=tb[:], in_=ps[:], func=COPY)
                    nc.sync.dma_start(
                        out=table[128 * b0:128 * (b0 + 4), :]
                        .rearrange("(k p) f -> p k f", k=4),
                        in_=tb[:])

            # ---- phase 2 + interleaved phase 3 ----
            with tc.tile_pool(name="p2g", bufs=6) as p2g, \
                 tc.tile_pool(name="p2gs", bufs=3) as p2gs, \
                 tc.tile_pool(name="p2e", bufs=2) as p2e, \
                 tc.tile_pool(name="p2s", bufs=3) as p2s, \
                 tc.tile_pool(name="p2t4", bufs=2) as p2t4, \
                 tc.tile_pool(name="p3s", bufs=2) as p3s, \
                 tc.tile_pool(name="p3o", bufs=2) as p3o, \
                 tc.tile_pool(name="pp1", bufs=2, space="PSUM") as pp1, \
                 tc.tile_pool(name="pp2", bufs=2, space="PSUM") as pp2, \
                 tc.tile_pool(name="pp3", bufs=2, space="PSUM") as pp3, \
                 tc.tile_pool(name="pp4", bufs=1, space="PSUM") as pp4:

                def ln1_block(b):
                    # aggT block -> bf16, mm3 (+48*mb3), LN1, h1 row-major + T
                    nc.scalar.activation(
                        out=s_aggTb[:, 128 * b:128 * (b + 1)],
                        in_=s_aggT[:, 128 * b:128 * (b + 1)], func=COPY)
                    psm = pp3.tile([128, 128], F32, tag="p3t")
                    nc.tensor.matmul(out=psm[:], lhsT=s_w3[:],
                                     rhs=s_aggTb[:, 128 * b:128 * (b + 1)],
                                     start=True, stop=True)
                    nc.vector.tensor_scalar_add(
                        out=s_a2Tb[:, 128 * b:128 * (b + 1)],
                        in0=psm[:], scalar1=s_mb3x48[:])
                    psrm = pp3.tile([128, 128], F32, tag="p3t")
                    nc.tensor.matmul(out=psrm[:],
                                     lhsT=s_a2Tb[:, 128 * b:128 * (b + 1)],
                                     rhs=s_id[:], start=True, stop=True)
                    x1 = p3s.tile([128, 128], F32, tag="x1")
                    nc.vector.tensor_add(out=x1[:], in0=psrm[:],
                                         in1=s_nhl[:, b, :])
                    st = p3s.tile([128, 6], F32, tag="st")
                    nc.vector.bn_stats(out=st[:], in_=x1[:])
                    mv = p3s.tile([128, 2], F32, tag="mv")
                    nc.vector.bn_aggr(out=mv[:], in_=st[:])
                    sd = p3s.tile([128, 1], F32, tag="sd")
                    nc.scalar.activation(out=sd[:], in_=mv[:, 1:2], func=SQRT,
                                         bias=s_eps[:])
                    rstd = p3s.tile([128, 1], F32, tag="rstd")
                    nc.vector.reciprocal(out=rstd[:], in_=sd[:])
                    xn = p3s.tile([128, 128], BF, tag="xn")
                    nc.vector.tensor_scalar(out=xn[:], in0=x1[:],
                                            scalar1=mv[:, 0:1], scalar2=rstd[:],
                                            op0=SUB, op1=MUL)
                    tb1 = p3s.tile([128, 128], BF, tag="tb1")
                    nc.vector.tensor_mul(out=tb1[:], in0=xn[:], in1=s_g1bc[:])
                    nc.vector.tensor_add(out=s_h1rm[:, b, :], in0=tb1[:],
                                         in1=s_b1bc[:])
                    psT = pp3.tile([128, 128], F32, tag="p3t")
                    nc.tensor.matmul(out=psT[:], lhsT=s_h1rm[:, b, :],
                                     rhs=s_id[:], start=True, stop=True)
                    nc.scalar.activation(out=s_h1T[:, 128 * b:128 * (b + 1)],
                                         in_=psT[:], func=COPY)

                def ff_half(hh):
                    us = []
                    for fc in range(4):
                        psf = pp4.tile([128, hw_], F32, tag="psf")
                        nc.tensor.matmul(out=psf[:],
                                         lhsT=s_fw1[:, 128 * fc:128 * (fc + 1)],
                                         rhs=s_h1T[:, hw_ * hh:hw_ * (hh + 1)],
                                         start=True, stop=True)
                        u = p3s.tile([128, hw_], BF, tag=f"u{fc}")
                        nc.scalar.activation(out=u[:], in_=psf[:], func=GELU,
                                             bias=s_fb1c[:, fc:fc + 1])
                        us.append(u)
                    psf2 = pp4.tile([128, hw_], F32, tag="psf2")
                    for fc in range(4):
                        nc.tensor.matmul(out=psf2[:], lhsT=s_fw2[:, fc, :],
                                         rhs=us[fc][:], start=(fc == 0),
                                         stop=(fc == 3))
                    obh = p3o.tile([128, blk_h, 128], F32, tag="obh")
                    for j in range(blk_h):
                        b = blk_h * hh + j
                        ffT = p3s.tile([128, 128], BF, tag="ffT")
                        nc.vector.tensor_scalar_add(
                            out=ffT[:], in0=psf2[:, 128 * j:128 * (j + 1)],
                            scalar1=s_fb2c[:])
                        psr2 = pp3.tile([128, 128], F32, tag="p3t")
                        nc.tensor.matmul(out=psr2[:], lhsT=ffT[:], rhs=s_id[:],
                                         start=True, stop=True)
                        ffrm = p3s.tile([128, 128], BF, tag="ffrm")
                        nc.scalar.activation(out=ffrm[:], in_=psr2[:],
                                             func=COPY)
                        x2 = p3s.tile([128, 128], F32, tag="x2")
                        nc.vector.tensor_add(out=x2[:], in0=ffrm[:],
                                             in1=s_h1rm[:, b, :])
                        st2 = p3s.tile([128, 6], F32, tag="st2")
                        nc.vector.bn_stats(out=st2[:], in_=x2[:])
                        mv2 = p3s.tile([128, 2], F32, tag="mv2")
                        nc.vector.bn_aggr(out=mv2[:], in_=st2[:])
                        sd2 = p3s.tile([128, 1], F32, tag="sd2")
                        nc.scalar.activation(out=sd2[:], in_=mv2[:, 1:2],
                                             func=SQRT, bias=s_eps[:])
                        rstd2 = p3s.tile([128, 1], F32, tag="rstd2")
                        nc.vector.reciprocal(out=rstd2[:], in_=sd2[:])
                        xn2 = p3s.tile([128, 128], F32, tag="xn2")
                        nc.vector.tensor_scalar(out=xn2[:], in0=x2[:],
                                                scalar1=mv2[:, 0:1],
                                                scalar2=rstd2[:],
                                                op0=SUB, op1=MUL)
                        tg = p3s.tile([128, 128], F32, tag="tg")
                        nc.vector.tensor_mul(out=tg[:], in0=xn2[:],
                                             in1=s_g2bc[:])
                        nc.vector.tensor_add(out=obh[:, j, :], in0=tg[:],
                                             in1=s_b2bc[:])
                    nc.sync.dma_start(
                        out=out.ap()[hw_ * hh:hw_ * (hh + 1), :]
                        .rearrange("(j p) f -> p j f", j=blk_h),
                        in_=obh[:])

                for ch in range(N_CH):
                    g = p2g.tile([128, sub_ch, 256], F8, tag="g")
                    nc.gpsimd.dma_gather(
                        out_ap=g[:],
                        in_ap=table[:],
                        idxs_ap=s_idx[:, (ch_rows // 16) * ch:
                                      (ch_rows // 16) * (ch + 1)],
                        num_idxs=ch_rows,
                        num_idxs_reg=ch_rows,
                        elem_size=256,
                        single_packet=False,
                        queue_num=ch % NQ,
                    )
                    e = p2e.tile([128, ch_rows], F8, tag="e")
                    nc.scalar.dma_start(out=e[:],
                                        in_=edge_hT.ap()[:, ch_rows * ch:
                                                         ch_rows * (ch + 1)])
                    # seq half *= ar_mask (one strided broadcast mul)
                    m = s_maskc[:, sub_ch * ch:sub_ch * (ch + 1)]
                    mb = bass.AP(tensor=m.tensor, offset=m.offset,
                                 ap=[m.ap[0], m.ap[1], [0, 128]])
                    gs = p2gs.tile([128, sub_ch, 128], BF, tag="gs")
                    nc.vector.tensor_mul(out=gs[:], in0=g[:, :, 128:256],
                                         in1=mb)
                    # += node half (single bf16 inject per sub)
                    nc.vector.tensor_add(out=gs[:], in0=gs[:],
                                         in1=g[:, :, 0:128])
                    t4c = p2t4.tile([128, 8 * cht, KK], BF, tag="t4c")
                    for tt in range(cht):
                        t = ch * cht + tt
                        ps1 = pp1.tile([128, RT], F32, tag="ps1")
                        nc.tensor.matmul(
                            out=ps1[:],
                            lhsT=s_wc[:],
                            rhs=e[:, RT * tt:RT * (tt + 1)],
                            start=True, stop=False)
                        nb = s_nhTl[:, 8 * t:8 * t + 8]
                        rep = bass.AP(tensor=nb.tensor, offset=nb.offset,
                                      ap=[nb.ap[0], nb.ap[1], [0, KK]])
                        nc.tensor.matmul(out=ps1[:], lhsT=s_wa[:], rhs=rep,
                                         start=False, stop=False)
                        for c in range(3):
                            nc.tensor.matmul(out=ps1[:, 128 * c:128 * (c + 1)],
                                             lhsT=gs[:, 3 * tt + c, :],
                                             rhs=s_id[:],
                                             start=False, stop=(c == 2))
                        t2 = p2s.tile([128, RT], BF, tag="t2")
                        nc.scalar.activation(out=t2[:], in_=ps1[:], func=GELU,
                                             bias=s_mb1c[:])
                        ps3 = pp2.tile([128, RT], F32, tag="ps3")
                        nc.tensor.matmul(out=ps3[:], lhsT=s_w2[:], rhs=t2[:],
                                         start=True, stop=True)
                        nc.scalar.activation(
                            out=t4c[:, 8 * tt:8 * (tt + 1), :], in_=ps3[:],
                            func=GELU, bias=s_mb2c[:])
                    # one k-sum per chunk: [128, nodes, 48] -> [128, nodes]
                    nc.vector.reduce_sum(
                        out=s_aggT[:, 8 * cht * ch:8 * cht * (ch + 1)],
                        in_=t4c[:], axis=AX)
                    # FFN + LN2 + output as soon as a half is complete
                    if (ch + 1) % blk_h == 0:
                        hh = (ch + 1) // blk_h - 1
                        for j in range(blk_h):
                            ln1_block(blk_h * hh + j)
                        ff_half(hh)

    nc.compile()
    return nc


def prep_core_inputs(inputs, n_glob, n_loc, core):
    """Host-side layout prep for one core. Pure layout/slicing/dtype casts +
    tiny constant broadcasts; no kernel math is done on the host."""
    f32 = np.float32
    n0 = core * n_loc
    E = n_loc * KK
    T = E // RT
    eh = np.ascontiguousarray(
        inputs["edge_h"][n0:n0 + n_loc].reshape(E, H).T).astype(F8NP)
    # Rotate the global node axis so this core's local nodes come first;
    # gather indices are rotated to match (table row r = global node
    # (n0 + r) % n_glob).
    j = (inputs["edge_idx"][n0:n0 + n_loc].reshape(E) - n0) % n_glob
    idx16 = np.ascontiguousarray(j.reshape(E // 16, 16).T).astype(np.int16)
    m = inputs["ar_mask"][n0:n0 + n_loc].reshape(E)
    maskc = np.ascontiguousarray(m.reshape(3 * T, 128).T).astype(f32)
    node_hT = np.ascontiguousarray(
        np.roll(inputs["node_h"], -n0, axis=0).T).astype(F8NP)
    seqT = np.ascontiguousarray(
        np.roll(inputs["seq_emb"], -n0, axis=0).T).astype(F8NP)
    nhl = np.ascontiguousarray(
        inputs["node_h"][n0:n0 + n_loc].reshape(n_loc // 128, 128, H)
        .transpose(1, 0, 2)).astype(f32)
    nhTl = np.ascontiguousarray(
        inputs["node_h"][n0:n0 + n_loc].T).astype(BF16)
    mW1 = inputs["mW1"]
    d = {
        "edge_hT": eh, "idx16": idx16, "maskc": maskc,
        "node_hT": node_hT, "seqT": seqT, "nhl": nhl, "nhTl": nhTl,
        "wa": mW1[0:128].astype(BF16), "wb": mW1[128:256].astype(BF16),
        "wc": mW1[256:384].astype(BF16),
        "wd": mW1[384:512].astype(BF16),
        "w2": inputs["mW2"].astype(BF16), "w3": inputs["mW3"].astype(BF16),
        "fw1": inputs["fW1"].astype(BF16),
        "fw2": np.ascontiguousarray(
            inputs["fW2"].reshape(4, 128, H).transpose(1, 0, 2)).astype(BF16),
        "ident": np.eye(128, dtype=BF16),
        "mb1c": inputs["mb1"].reshape(H, 1).astype(f32),
        "mb2c": inputs["mb2"].reshape(H, 1).astype(f32),
        "mb3x48": (inputs["mb3"] * KK).reshape(H, 1).astype(f32),
        "fb1c": np.ascontiguousarray(
            inputs["fb1"].reshape(4, 128).T).astype(f32),
        "fb2c": inputs["fb2"].reshape(H, 1).astype(f32),
        "g1bc": np.tile(inputs["g1"][None, :], (128, 1)).astype(BF16),
        "b1bc": np.tile(inputs["b1"][None, :], (128, 1)).astype(BF16),
        "g2bc": np.tile(inputs["g2"][None, :], (128, 1)).astype(f32),
        "b2bc": np.tile(inputs["b2"][None, :], (128, 1)).astype(f32),
    }
    return d


_NC_CACHE = {}


def kernel(**inputs):
    inputs = {k: np.asarray(v) for k, v in inputs.items()}
    n_glob = inputs["node_h"].shape[0]
    n_loc = n_glob // N_CORES
    key = (n_glob, n_loc)
    if key not in _NC_CACHE:
        _NC_CACHE[key] = build_nc(n_glob, n_loc, N_CORES)
    nc = _NC_CACHE[key]
    in_maps = [prep_core_inputs(inputs, n_glob, n_loc, c)
               for c in range(N_CORES)]
    res = bass_utils.run_bass_kernel_spmd(nc, in_maps,
                                          core_ids=list(range(N_CORES)))
    return np.concatenate([res.results[c]["out"] for c in range(N_CORES)],
                          axis=0).astype(np.float32)
